# revision 1
# baseline (speedup 1.0000x reference)
"""Trainium2 Bass kernel v2 for nn_CognitiveAttention (B=4, S=2048, H=768, NH=12).

Same sharding as baseline: 8 cores = (batch, seq-half); each core handles
1024 queries x compacted keys (skp) x 12 heads. Zero cross-core comms.

v2 moves all matmuls to fp8 (e4m3):
  - Q/K/V/out projections + attn*V use DoubleRow perf mode (2 k-blocks per
    instruction, 0.5 PE cycles/row -> 2x throughput over fp32r).
  - scores use normal fp8 (contraction=64, same cycles as fp32r, smaller SBUF).
  - ACT engine runs ONLY exp (the bottleneck: ~92us of the ~105us target);
    all psum->SBUF converts ride DVE/Pool, LN split across DVE/Pool/PE.

Scaling scheme (host pre-scales, device rescales in converts):
  hs8 = fp8(16*hs), w8 = fp8(64*W)
  kT8 = fp8(16*k) = fp8(psum/64 + 16*bk)      qT8 = fp8(2*q) = fp8(psum/512 + 2*bq)
  scores psum = sum(qT8*kT8) = 256*s          pT8 = fp8(exp(s)/16)  [exp bias ln(1/16)]
  v8 = fp8(16*v)*mask, ones col = 0.25*mask   -> num/den = 64*ctx exactly
  ctx8 = fp8(64*ctx)                          out = psum/4096 + (bo + Wo@bv)
  bv folded into bo on host (rows of softmax sum to 1).
"""

import numpy as np

import concourse.bass as bass
import concourse.tile as tile
from concourse import bacc, mybir
from concourse.bass_utils import run_bass_kernel_spmd
from concourse.masks import make_identity

F32 = mybir.dt.float32
F8 = mybir.dt.float8e4
BF16 = mybir.dt.bfloat16
AF = mybir.ActivationFunctionType
OP = mybir.AluOpType
DR = mybir.MatmulPerfMode.DoubleRow

H = 768
NH = 12
HD = 64
SQ = 1024
N_CORES = 8
LN_EPS = 1e-5
LN_SP = float(np.log(1.0 / 16.0))   # exp output scale folded into bias

_CACHE = {}


def _build(skp, repeat=1):
    nbk = skp // 128
    nc = bacc.Bacc("TRN2", target_bir_lowering=False, debug=False,
                   num_devices=N_CORES)

    hsT_kv_d = nc.dram_tensor("hsT_kv8", [H, skp], F8, kind="ExternalInput")
    hsT_q_d = nc.dram_tensor("hsT_q8", [H, SQ], F8, kind="ExternalInput")
    hs_q_d = nc.dram_tensor("hs_q", [SQ, H], BF16, kind="ExternalInput")
    wqT_d = nc.dram_tensor("wqT8", [H, H], F8, kind="ExternalInput")
    wkT_d = nc.dram_tensor("wkT8", [H, H], F8, kind="ExternalInput")
    wvT_d = nc.dram_tensor("wvT8", [H, H], F8, kind="ExternalInput")
    woT_d = nc.dram_tensor("woT8", [H, H], F8, kind="ExternalInput")
    bq2_d = nc.dram_tensor("bq2", [128, 6], F32, kind="ExternalInput")
    bk2_d = nc.dram_tensor("bk2", [128, 6], F32, kind="ExternalInput")
    bo2_d = nc.dram_tensor("bo2", [128, 6], F32, kind="ExternalInput")
    m01_d = nc.dram_tensor("m01", [128, nbk], F32, kind="ExternalInput")
    gam_d = nc.dram_tensor("gam", [1, H], F32, kind="ExternalInput")
    bet_d = nc.dram_tensor("bet", [1, H], F32, kind="ExternalInput")
    y_d = nc.dram_tensor("y_out", [SQ, H], F32, kind="ExternalOutput")

    npair = nbk // 2          # DoubleRow key-block pairs
    ktail = nbk % 2

    with tile.TileContext(nc) as tc:
      for _rep in range(repeat):
        with tc.tile_pool(name="persist", bufs=1) as pp, \
             tc.tile_pool(name="psS", bufs=2, space="PSUM") as pss, \
             tc.tile_pool(name="psC", bufs=1, space="PSUM") as psc, \
             tc.tile_pool(name="psX", bufs=1, space="PSUM") as psx:
            # ---- persistent SBUF tiles ----
            hskv8 = pp.tile([128, 6, skp], F8)
            hsq8 = pp.tile([128, 6, SQ], F8)
            wq8 = pp.tile([128, 6, H], F8)
            wk8 = pp.tile([128, 6, H], F8)
            wv8 = pp.tile([128, 6, H], F8)
            wo8 = pp.tile([128, 6, H], F8)
            kT8 = pp.tile([128, 6, skp], BF16)
            qT8 = pp.tile([128, 6, SQ], BF16)
            v_pad8 = pp.tile([128, nbk, (NH // 2) * 192], F8)
            ctxT8 = pp.tile([128, 6, SQ], F8)
            outT = pp.tile([128, 6, SQ], BF16)
            hs_res = pp.tile([128, 8, H], BF16)
            bq2 = pp.tile([128, 6], F32)
            bk2 = pp.tile([128, 6], F32)
            bo2 = pp.tile([128, 6], F32)
            m01 = pp.tile([128, nbk], F32)
            gam = pp.tile([128, H], F32)
            bet = pp.tile([128, H], F32)
            ones384 = pp.tile([128, 384], F32)
            epsb = pp.tile([128, 1], F32)
            spb = pp.tile([128, 1], F32)
            zb = pp.tile([128, 1], F32)
            identb = pp.tile([128, 128], BF16)

            # ---- DMAs, priority order; split across SP and Pool queues ----
            nc.sync.dma_start(
                wk8[:], wkT_d.ap()[:].rearrange("(j p) c -> p j c", p=128))
            nc.gpsimd.dma_start(
                hskv8[:], hsT_kv_d.ap()[:].rearrange("(j p) c -> p j c", p=128))
            nc.sync.dma_start(
                wq8[:], wqT_d.ap()[:].rearrange("(j p) c -> p j c", p=128))
            nc.gpsimd.dma_start(
                hsq8[:], hsT_q_d.ap()[:].rearrange("(j p) c -> p j c", p=128))
            nc.sync.dma_start(
                wv8[:], wvT_d.ap()[:].rearrange("(j p) c -> p j c", p=128))
            nc.gpsimd.dma_start(
                wo8[:], woT_d.ap()[:].rearrange("(j p) c -> p j c", p=128))
            nc.sync.dma_start(bk2[:], bk2_d.ap()[:])
            nc.sync.dma_start(bq2[:], bq2_d.ap()[:])
            nc.sync.dma_start(m01[:], m01_d.ap()[:])
            nc.sync.dma_start(bo2[:], bo2_d.ap()[:])
            nc.gpsimd.dma_start(
                hs_res[:], hs_q_d.ap()[:].rearrange("(t p) c -> p t c", p=128))
            nc.gpsimd.dma_start(
                gam[:], bass.AP(tensor=gam_d, offset=0, ap=[(0, 128), (1, H)]))
            nc.gpsimd.dma_start(
                bet[:], bass.AP(tensor=bet_d, offset=0, ap=[(0, 128), (1, H)]))
            nc.vector.memset(ones384[:], 1.0)
            nc.vector.memset(epsb[:], LN_EPS)
            nc.vector.memset(spb[:], LN_SP)
            nc.vector.memset(zb[:], 0.0)
            make_identity(nc, identb[:])

            kchunks = []
            off = 0
            while off < skp:
                cw = min(512, skp - off)
                kchunks.append((off, cw))
                off += cw

            def kproj_big(m):
                """K-proj m-tile as one 3-bank round on the scores pool
                (prologue only, before the exp stream starts)."""
                ps = pss.tile([128, 3, 512], F32, tag="sT", name=f"pskb{m}")
                for ci_, (off, cw) in enumerate(kchunks):
                    for j in range(3):
                        nc.tensor.matmul(
                            ps[:, ci_, :cw],
                            wk8[:, 2 * j:2 * j + 2, m * 128:(m + 1) * 128],
                            hskv8[:, 2 * j:2 * j + 2, off:off + cw],
                            start=(j == 0), stop=(j == 2), perf_mode=DR)
                for ci_, (off, cw) in enumerate(kchunks):
                    nc.vector.tensor_scalar(
                        out=kT8[:, m, off:off + cw], in0=ps[:, ci_, :cw],
                        scalar1=1.0 / 64.0, scalar2=bk2[:, m:m + 1],
                        op0=OP.mult, op1=OP.add)

            def qproj_big(m):
                ps = pss.tile([128, 3, 512], F32, tag="sT", name=f"psqb{m}")
                for c in range(2):
                    for j in range(3):
                        nc.tensor.matmul(
                            ps[:, c, :],
                            wq8[:, 2 * j:2 * j + 2, m * 128:(m + 1) * 128],
                            hsq8[:, 2 * j:2 * j + 2, c * 512:(c + 1) * 512],
                            start=(j == 0), stop=(j == 2), perf_mode=DR)
                nc.vector.tensor_scalar(
                    out=qT8[:, m, :], in0=ps[:, 0:2, :],
                    scalar1=1.0 / 512.0, scalar2=bq2[:, m:m + 1],
                    op0=OP.mult, op1=OP.add)

            def push_kproj(m, conv):
                for (off, cw) in kchunks:
                    def fn(m=m, off=off, cw=cw, conv=conv):
                        ps = psx.tile([128, 512], F32, tag="px",
                                      name=f"psk{m}_{off}")
                        for j in range(3):
                            nc.tensor.matmul(
                                ps[:, :cw],
                                wk8[:, 2 * j:2 * j + 2, m * 128:(m + 1) * 128],
                                hskv8[:, 2 * j:2 * j + 2, off:off + cw],
                                start=(j == 0), stop=(j == 2), perf_mode=DR)
                        conv.tensor_scalar(
                            out=kT8[:, m, off:off + cw], in0=ps[:, :cw],
                            scalar1=1.0 / 64.0, scalar2=bk2[:, m:m + 1],
                            op0=OP.mult, op1=OP.add)
                    pending.append((f"k{m}_{off}", fn))

            def push_qproj(m, conv):
                for c in range(2):
                    def fn(m=m, c=c, conv=conv):
                        co = c * 512
                        ps = psx.tile([128, 512], F32, tag="px",
                                      name=f"psq{m}_{c}")
                        for j in range(3):
                            nc.tensor.matmul(
                                ps[:],
                                wq8[:, 2 * j:2 * j + 2, m * 128:(m + 1) * 128],
                                hsq8[:, 2 * j:2 * j + 2, co:co + 512],
                                start=(j == 0), stop=(j == 2), perf_mode=DR)
                        conv.tensor_scalar(
                            out=qT8[:, m, co:co + 512], in0=ps[:],
                            scalar1=1.0 / 512.0, scalar2=bq2[:, m:m + 1],
                            op0=OP.mult, op1=OP.add)
                    pending.append((f"kq{m}" if c == 1 else f"q{m}_{c}", fn))

            pv0 = v_pad8[:].ap[0]
            vrow = (NH // 2) * 192

            def push_vproj(tb, ci, conv, label=None):
                def fn(tb=tb, ci=ci, conv=conv):
                    ps = psx.tile([128, 384], F32, tag="px",
                                  name=f"psv{tb}_{ci}")
                    for j in range(3):
                        nc.tensor.matmul(
                            ps[:],
                            hskv8[:, 2 * j:2 * j + 2, tb * 128:(tb + 1) * 128],
                            wv8[:, 2 * j:2 * j + 2, ci * 384:(ci + 1) * 384],
                            start=(j == 0), stop=(j == 2), perf_mode=DR)
                    dst = bass.AP(
                        tensor=v_pad8.tensor,
                        offset=v_pad8[:].offset + tb * vrow + ci * 576,
                        ap=[pv0, (192, 3), (128, 2), (1, 64)])
                    conv.tensor_scalar(
                        out=dst, in0=ps[:],
                        scalar1=m01[:, tb:tb + 1], scalar2=1.0 / 64.0,
                        op0=OP.mult, op1=OP.mult)
                pending.append((label or f"v{tb}_{ci}", fn))

            def vones(tb, conv):
                ones_dst = bass.AP(
                    tensor=v_pad8.tensor,
                    offset=v_pad8[:].offset + tb * vrow + 64,
                    ap=[pv0, (192, 6), (1, 64)])
                # ones col = 0.25 * m01
                conv.tensor_scalar(
                    out=ones_dst, in0=ones384[:],
                    scalar1=m01[:, tb:tb + 1], scalar2=0.25,
                    op0=OP.mult, op1=OP.mult)

            pT_tiles = {}
            pending = []          # queue of deferred psx rounds (closures)
            drained = set()       # labels fully drained

            def drain(n):
                k = 0
                while pending and k < n:
                    label, fn = pending.pop(0)
                    fn()
                    drained.add(label)
                    k += 1

            def drain_until(label):
                while pending and label not in drained:
                    lb, fn = pending.pop(0)
                    fn()
                    drained.add(lb)

            _slot = [0]

            def sc_exp(h, c):
                """scores + exp for (head, 512-query chunk) -> pT8 tile."""
                hj = h // 2
                po = (h % 2) * 64
                co = c * 512
                drain_until(f"kq{hj}")
                pT = pss_sb.tile([128, nbk, 512], F8, tag="pT",
                                 name=f"pT{h}_{c}")
                pT_tiles[(h, c)] = pT
                for g0 in range(0, nbk, 3):
                    g1 = min(g0 + 3, nbk)
                    ps = pss.tile([128, 3, 512], F32, tag="sT",
                                  name=f"sT{h}_{c}_{g0}")
                    for i in range(g0, g1):
                        nc.tensor.matmul(
                            ps[:, i - g0, :],
                            kT8[po:po + 64, hj, i * 128:(i + 1) * 128],
                            qT8[po:po + 64, hj, co:co + 512])
                    nc.scalar.activation(
                        pT[:, g0:g1, :], ps[:, 0:g1 - g0, :], AF.Exp,
                        scale=1.0 / 256.0, bias=spb[:])
                    _slot[0] += 1
                    drain(2 if _slot[0] % 3 == 0 else 1)

            def ctx_head(h, c):
                """attn*V (rowsum via masked 0.25-ones col) + normalize."""
                hj = h // 2
                po = (h % 2) * 64
                co = c * 512
                drain_until(f"vci{0 if h < 6 else 1}")
                pT = pT_tiles.pop((h, c))
                vco = hj * 192 + po
                cps = psc.tile([128, 512], F32, tag="cT", name=f"cT{h}_{c}")
                for i2 in range(npair):
                    nc.tensor.matmul(
                        cps[:], v_pad8[:, 2 * i2:2 * i2 + 2, vco:vco + 128],
                        pT[:, 2 * i2:2 * i2 + 2, :],
                        start=(i2 == 0), stop=(ktail == 0 and i2 == npair - 1),
                        perf_mode=DR)
                if ktail:
                    nc.tensor.matmul(
                        cps[:], v_pad8[:, nbk - 1, vco:vco + 128],
                        pT[:, nbk - 1, :], start=(npair == 0), stop=True)
                rs = rsp.tile([128, 512], F32, tag="rs", name=f"rs{h}_{c}")
                nc.vector.reciprocal(rs[po:po + 64, :],
                                     cps[64 - po:128 - po, :])
                nc.vector.tensor_tensor(
                    out=ctxT8[po:po + 64, hj, co:co + 512],
                    in0=cps[po:po + 64, :], in1=rs[po:po + 64, :],
                    op=OP.mult)

            def push_outproj(m, c, conv):
                def fn(m=m, c=c, conv=conv):
                    co = c * 512
                    ps = psx.tile([128, 512], F32, tag="px", name=f"pso{m}_{c}")
                    for j in range(3):
                        nc.tensor.matmul(
                            ps[:],
                            wo8[:, 2 * j:2 * j + 2, m * 128:(m + 1) * 128],
                            ctxT8[:, 2 * j:2 * j + 2, co:co + 512],
                            start=(j == 0), stop=(j == 2), perf_mode=DR)
                    conv.tensor_scalar(
                        out=outT[:, m, co:co + 512], in0=ps[:],
                        scalar1=1.0 / 4096.0, scalar2=bo2[:, m:m + 1],
                        op0=OP.mult, op1=OP.add)
                pending.append((f"op{c}_{m}", fn))

            def push_ln(tb):
                def fn(tb=tb):
                    ln_tb(tb)
                pending.append((f"ln{tb}", fn))

            def ln_tb(tb):
                pt = psx.tile([128, 6, 128], BF16, tag="px", name=f"pt{tb}")
                for m in range(6):
                    nc.tensor.transpose(
                        pt[:, m, :], outT[:, m, tb * 128:(tb + 1) * 128],
                        identb[:])
                y = pdl.tile([128, H], F32, tag="y", name=f"y{tb}")
                # pt is PSUM: GPSIMD cannot read PSUM, this add must be DVE
                nc.vector.tensor_tensor(
                    out=y[:], in0=pt[:].rearrange("p a b -> p (a b)"),
                    in1=hs_res[:, tb, :], op=OP.add)
                stats = pdl.tile([128, 3, 6], F32, tag="st", name=f"st{tb}")
                yv = y[:].rearrange("p (n f) -> p n f", f=256)
                for g in range(3):
                    nc.vector.bn_stats(out=stats[:, g, :], in_=yv[:, g, :])
                mv = pdl.tile([128, 2], F32, tag="mv", name=f"mv{tb}")
                nc.vector.bn_aggr(out=mv[:], in_=stats[:])
                rstd = pdl.tile([128, 1], F32, tag="rstd", name=f"rstd{tb}")
                # rstd = exp(-0.5*ln(var+eps)). On HW, exp and ln share the
                # natural_log_exp_and_others ACT table set, so this never
                # reloads tables against the exp stream (Sqrt would).
                nc.scalar.activation(rstd[:], mv[:, 1:2], AF.Ln, bias=epsb[:])
                nc.scalar.activation(rstd[:], rstd[:], AF.Exp, scale=-0.5,
                                     bias=zb[:])
                nmr = pdl.tile([128, 1], F32, tag="nmr", name=f"nmr{tb}")
                nc.vector.scalar_tensor_tensor(
                    out=nmr[:], in0=mv[:, 0:1], scalar=-1.0, in1=rstd[:],
                    op0=OP.mult, op1=OP.mult)
                yn = pdl.tile([128, H], F32, tag="yn", name=f"yn{tb}")
                # SBUF-only affine tail: alternate Pool/DVE/Pool to spread load
                nc.gpsimd.tensor_scalar(
                    out=yn[:], in0=y[:], scalar1=rstd[:], scalar2=nmr[:],
                    op0=OP.mult, op1=OP.add)
                nc.vector.tensor_tensor(out=yn[:], in0=yn[:], in1=gam[:],
                                        op=OP.mult)
                nc.gpsimd.tensor_tensor(out=yn[:], in0=yn[:], in1=bet[:],
                                        op=OP.add)
                nc.sync.dma_start(y_d.ap()[tb * 128:(tb + 1) * 128, :], yn[:])

            with tc.tile_pool(name="pTp", bufs=14) as pss_sb, \
                 tc.tile_pool(name="rsP", bufs=2) as rsp, \
                 tc.tile_pool(name="phD", bufs=2) as pdl:
                # ---- prologue: K0/Q0 as big rounds on the scores pool so
                # the exp stream starts ASAP ----
                kproj_big(0)
                qproj_big(0)
                drained.add("kq0")
                for tb in range(nbk):
                    vones(tb, nc.gpsimd)
                # ---- deferred psx rounds, drained ~1.33/score-group into
                # the gaps of the exp stream; K/Q first (hard deadlines),
                # then V (ctx is deferred until its V half is written) ----
                push_kproj(1, nc.vector)
                push_qproj(1, nc.vector)
                push_kproj(2, nc.vector)
                push_qproj(2, nc.vector)
                for tb in range(nbk):
                    push_vproj(tb, 0, nc.vector,
                               label=("vci0" if tb == nbk - 1 else None))
                push_kproj(3, nc.vector)
                push_qproj(3, nc.vector)
                push_kproj(4, nc.vector)
                push_qproj(4, nc.vector)
                push_kproj(5, nc.vector)
                push_qproj(5, nc.vector)
                for tb in range(nbk):
                    push_vproj(tb, 1, nc.vector,
                               label=("vci1" if tb == nbk - 1 else None))

                order = [(h, 0) for h in range(NH)] + [(h, 1) for h in range(NH)]
                ctx_todo = list(order)

                def ctx_ready(h, c):
                    return (f"vci{0 if h < 6 else 1}" in drained
                            and (h, c) in pT_tiles)

                op0_done = False
                for (h, c) in order:
                    sc_exp(h, c)
                    for _ in range(2):
                        if ctx_todo and ctx_ready(*ctx_todo[0]):
                            ctx_head(*ctx_todo.pop(0))
                        else:
                            break
                    if not op0_done and not any(cc == 0 for (_, cc) in ctx_todo):
                        for m in range(6):
                            push_outproj(m, 0, nc.vector)
                        for tb in range(4):
                            push_ln(tb)
                        op0_done = True
                # ---- tail ----
                while ctx_todo:
                    ctx_head(*ctx_todo.pop(0))
                drain(len(pending))
                for m in range(6):
                    push_outproj(m, 1, nc.vector)
                for tb in range(4, 8):
                    push_ln(tb)
                drain(len(pending))

    nc.compile()
    return nc


def _make_in_maps(inputs, idxs, skp):
    import ml_dtypes
    F8NP = ml_dtypes.float8_e4m3
    BF16NP = ml_dtypes.bfloat16

    hs = np.ascontiguousarray(np.asarray(inputs["hidden_states"], np.float32))
    Wq, Wk, Wv, Wo = (np.asarray(inputs[k], np.float32)
                      for k in ("Wq", "Wk", "Wv", "Wo"))
    bq, bk, bv, bo = (np.asarray(inputs[k], np.float32)
                      for k in ("bq", "bk", "bv", "bo"))
    wqT8 = np.ascontiguousarray((64.0 * Wq.T).astype(F8NP))
    wkT8 = np.ascontiguousarray((64.0 * Wk.T).astype(F8NP))
    wvT8 = np.ascontiguousarray((64.0 * Wv.T).astype(F8NP))
    woT8 = np.ascontiguousarray((64.0 * Wo.T).astype(F8NP))
    bq2 = np.ascontiguousarray((2.0 * bq).reshape(6, 128).T)
    bk2 = np.ascontiguousarray((16.0 * bk).reshape(6, 128).T)
    bo_eff = bo + Wo @ bv
    bo2 = np.ascontiguousarray(bo_eff.reshape(6, 128).T)
    gam = np.asarray(inputs["ln_gamma"], np.float32).reshape(1, H)
    bet = np.asarray(inputs["ln_beta"], np.float32).reshape(1, H)

    in_maps = []
    for core in range(N_CORES):
        b, sh = divmod(core, 2)
        ix = idxs[b]
        hsk = np.zeros((skp, H), np.float32)
        hsk[:len(ix)] = hs[b][ix]
        m01 = np.zeros(skp, np.float32)
        m01[:len(ix)] = 1.0
        hq = hs[b, sh * SQ:(sh + 1) * SQ]
        in_maps.append({
            "hsT_kv8": np.ascontiguousarray((16.0 * hsk.T).astype(F8NP)),
            "hsT_q8": np.ascontiguousarray((16.0 * hq.T).astype(F8NP)),
            "hs_q": np.ascontiguousarray(hq.astype(BF16NP)),
            "wqT8": wqT8, "wkT8": wkT8, "wvT8": wvT8, "woT8": woT8,
            "bq2": bq2, "bk2": bk2, "bo2": bo2,
            "m01": np.ascontiguousarray(m01.reshape(skp // 128, 128).T),
            "gam": gam, "bet": bet,
        })
    return in_maps


def kernel(hidden_states, Wq, bq, Wk, bk, Wv, bv, Wo, bo, dim_biases,
           ln_gamma, ln_beta, attention_mask, dimension_idx):
    hs = np.asarray(hidden_states, dtype=np.float32)
    mask = np.asarray(attention_mask)
    B, S, _ = hs.shape

    idxs = [np.nonzero(mask[b] != 0)[0] for b in range(B)]
    skp = max(256, ((max(len(ix) for ix in idxs) + 127) // 128) * 128)

    if skp not in _CACHE:
        _CACHE[skp] = _build(skp)
    nc = _CACHE[skp]

    in_maps = _make_in_maps(
        {"hidden_states": hs, "Wq": Wq, "Wk": Wk, "Wv": Wv, "Wo": Wo,
         "bq": bq, "bk": bk, "bv": bv, "bo": bo,
         "ln_gamma": ln_gamma, "ln_beta": ln_beta}, idxs, skp)

    res = run_bass_kernel_spmd(nc, in_maps, list(range(N_CORES)))

    out = np.empty((B, S, H), np.float32)
    for core in range(N_CORES):
        b, sh = divmod(core, 2)
        out[b, sh * SQ:(sh + 1) * SQ] = res.results[core]["y_out"]
    return out



# revision 19
# speedup vs baseline: 1.1055x; 1.1055x over previous
"""Trainium2 Bass kernel v4 for nn_CognitiveAttention (B=4, S=2048, H=768, NH=12).

Sharding: 8 cores = (batch, seq-half); each core handles 1024 queries x
compacted keys (skp) x 12 heads. Zero cross-core comms.

v4 over v2 (same fp8 math/scaling scheme):
  - LN rstd on DVE via rsqrt bit-trick + 2 Newton steps, batched per tb
    pair. Kills the 17 ACT table loads (exp<->ln set ping-pong) v2 paid.
  - softmax denominators via reciprocal_approx_fast (1 custom DVE op).
  - startup: DMAs ordered by the exp stream's critical path (hsq8 first
    half -> qproj c=0; hskv8 -> kproj; wk/wq m=0 column slices first),
    and the prologue emits qproj(c=0) before kproj so the PE FIFO never
    head-of-line blocks on a late DMA. Exp starts ~7us earlier.
  - tail: c=1 heads processed 8..11 first so outproj's last accumulation
    step covers the last-finished quartet; outproj c=1 rounds rotate over
    three PSUM tiles with converts alternating DVE/ACT; tail LN stats via
    ACT Identity/Square accum passes (ACT is idle there); affine fused to
    2 wide ops, spread across DVE/Pool.

Scaling scheme (host pre-scales, device rescales in converts):
  hs8 = fp8(16*hs), w8 = fp8(64*W)
  kT8 = fp8(16*k) = fp8(psum/64 + 16*bk)      qT8 = fp8(2*q) = fp8(psum/512 + 2*bq)
  scores psum = sum(qT8*kT8) = 256*s          pT8 = fp8(exp(s)/16)  [exp bias ln(1/16)]
  v8 = fp8(16*v)*mask, ones col = 0.25*mask   -> num/den = 64*ctx exactly
  ctx8 = fp8(64*ctx)                          out = psum/4096 + (bo + Wo@bv)
  bv folded into bo on host (rows of softmax sum to 1).
"""

import numpy as np

import concourse.bass as bass
import concourse.tile as tile
from concourse import bacc, mybir
from concourse.bass_utils import run_bass_kernel_spmd
from concourse.masks import make_identity

F32 = mybir.dt.float32
I32 = mybir.dt.int32
F8 = mybir.dt.float8e4
BF16 = mybir.dt.bfloat16
AF = mybir.ActivationFunctionType
OP = mybir.AluOpType
DR = mybir.MatmulPerfMode.DoubleRow

H = 768
NH = 12
HD = 64
SQ = 1024
N_CORES = 8
LN_EPS = 1e-5
LN_SP = float(np.log(1.0 / 16.0))   # exp output scale folded into bias
# f32 whose bits are 0x5f3759df (rsqrt magic)
RSQRT_MAGIC_F = float(
    np.frombuffer(np.array([0x5F3759DF], dtype=np.uint32).tobytes(),
                  dtype=np.float32)[0])

_CACHE = {}


def _build(skp, repeat=1):
    nbk = skp // 128
    nc = bacc.Bacc("TRN2", target_bir_lowering=False, debug=False,
                   num_devices=N_CORES)

    hsT_kv_d = nc.dram_tensor("hsT_kv8", [H, skp], F8, kind="ExternalInput")
    hsT_q_d = nc.dram_tensor("hsT_q8", [H, SQ], F8, kind="ExternalInput")
    hs_q_d = nc.dram_tensor("hs_q", [SQ, H], BF16, kind="ExternalInput")
    wqT_d = nc.dram_tensor("wqT8", [H, H], F8, kind="ExternalInput")
    wkT_d = nc.dram_tensor("wkT8", [H, H], F8, kind="ExternalInput")
    wvT_d = nc.dram_tensor("wvT8", [H, H], F8, kind="ExternalInput")
    woT_d = nc.dram_tensor("woT8", [H, H], F8, kind="ExternalInput")
    bq2_d = nc.dram_tensor("bq2", [128, 6], F32, kind="ExternalInput")
    bk2_d = nc.dram_tensor("bk2", [128, 6], F32, kind="ExternalInput")
    bo2_d = nc.dram_tensor("bo2", [128, 6], F32, kind="ExternalInput")
    m01_d = nc.dram_tensor("m01", [128, nbk], F32, kind="ExternalInput")
    gam_d = nc.dram_tensor("gam", [1, H], F32, kind="ExternalInput")
    bet_d = nc.dram_tensor("bet", [1, H], F32, kind="ExternalInput")
    y_d = nc.dram_tensor("y_out", [SQ, H], F32, kind="ExternalOutput")

    npair = nbk // 2          # DoubleRow key-block pairs
    ktail = nbk % 2

    with tile.TileContext(nc) as tc:
      for _rep in range(repeat):
        with tc.tile_pool(name="persist", bufs=1) as pp, \
             tc.tile_pool(name="psS", bufs=2, space="PSUM") as pss, \
             tc.tile_pool(name="psC", bufs=1, space="PSUM") as psc, \
             tc.tile_pool(name="psX", bufs=1, space="PSUM") as psx:
            # ---- persistent SBUF tiles ----
            hskv8 = pp.tile([128, 6, skp], F8)
            hsq8 = pp.tile([128, 6, SQ], F8)
            wq8 = pp.tile([128, 6, H], F8)
            wk8 = pp.tile([128, 6, H], F8)
            wv8 = pp.tile([128, 6, H], F8)
            wo8 = pp.tile([128, 6, H], F8)
            kT8 = pp.tile([128, 6, skp], BF16)
            qT8 = pp.tile([128, 6, SQ], BF16)
            v_pad8 = pp.tile([128, nbk, (NH // 2) * 192], F8)
            ctxT8 = pp.tile([128, 6, SQ], F8)
            outT = pp.tile([128, 6, SQ], BF16)
            hs_res = pp.tile([128, 8, H], BF16)
            bq2 = pp.tile([128, 6], F32)
            bk2 = pp.tile([128, 6], F32)
            bo2 = pp.tile([128, 6], F32)
            m01 = pp.tile([128, nbk], F32)
            gam = pp.tile([128, H], F32)
            bet = pp.tile([128, H], F32)
            ones384 = pp.tile([128, 384], F32)
            spb = pp.tile([128, 1], F32)
            magicf = pp.tile([128, 2], F32)
            mvs = pp.tile([128, 8, 2], F32)     # per-tb (mean, var)
            sms = pp.tile([128, 8, 2], F32)     # per-tb (sum, sumsq)
            rstds = pp.tile([128, 8], F32)      # per-tb 1/sqrt(var+eps)
            dump = pp.tile([128, H], F32)       # ACT accum-pass sink
            identb = pp.tile([128, 128], BF16)

            # ---- DMAs, ordered by the exp stream's critical path: the
            # first scores group needs kT chunk0 (hskv8 cols 0:512) and
            # qT c0 (hsq8 cols 0:512) only ----
            nc.gpsimd.dma_start(
                hskv8[:, :, 0:512],
                hsT_kv_d.ap()[:, 0:512].rearrange("(j p) c -> p j c", p=128))
            nc.sync.dma_start(
                wk8[:, :, 0:128],
                wkT_d.ap()[:, 0:128].rearrange("(j p) c -> p j c", p=128))
            nc.sync.dma_start(
                wq8[:, :, 0:128],
                wqT_d.ap()[:, 0:128].rearrange("(j p) c -> p j c", p=128))
            nc.sync.dma_start(bk2[:], bk2_d.ap()[:])
            nc.sync.dma_start(bq2[:], bq2_d.ap()[:])
            nc.gpsimd.dma_start(
                hsq8[:, :, 0:512],
                hsT_q_d.ap()[:, 0:512].rearrange("(j p) c -> p j c", p=128))
            nc.gpsimd.dma_start(
                hskv8[:, :, 512:skp],
                hsT_kv_d.ap()[:, 512:skp].rearrange("(j p) c -> p j c", p=128))
            nc.gpsimd.dma_start(
                hsq8[:, :, 512:SQ],
                hsT_q_d.ap()[:, 512:SQ].rearrange("(j p) c -> p j c", p=128))
            nc.sync.dma_start(
                wk8[:, :, 128:H],
                wkT_d.ap()[:, 128:H].rearrange("(j p) c -> p j c", p=128))
            nc.sync.dma_start(
                wq8[:, :, 128:H],
                wqT_d.ap()[:, 128:H].rearrange("(j p) c -> p j c", p=128))
            nc.sync.dma_start(m01[:], m01_d.ap()[:])
            nc.sync.dma_start(
                wv8[:], wvT_d.ap()[:].rearrange("(j p) c -> p j c", p=128))
            nc.gpsimd.dma_start(
                wo8[:], woT_d.ap()[:].rearrange("(j p) c -> p j c", p=128))
            nc.sync.dma_start(bo2[:], bo2_d.ap()[:])
            nc.gpsimd.dma_start(
                hs_res[:], hs_q_d.ap()[:].rearrange("(t p) c -> p t c", p=128))
            nc.gpsimd.dma_start(
                gam[:], bass.AP(tensor=gam_d, offset=0, ap=[(0, 128), (1, H)]))
            nc.gpsimd.dma_start(
                bet[:], bass.AP(tensor=bet_d, offset=0, ap=[(0, 128), (1, H)]))
            nc.vector.memset(ones384[:], 1.0)
            nc.vector.memset(spb[:], LN_SP)
            nc.vector.memset(magicf[:], RSQRT_MAGIC_F)
            make_identity(nc, identb[:])

            kchunks = []
            off = 0
            while off < skp:
                cw = min(512, skp - off)
                kchunks.append((off, cw))
                off += cw

            def kproj_big(m):
                """K-proj m-tile on the scores pool, converting each chunk
                as soon as its matmuls land (prologue only)."""
                ps = pss.tile([128, 3, 512], F32, tag="sT", name=f"pskb{m}")
                for ci_, (off, cw) in enumerate(kchunks):
                    for j in range(3):
                        nc.tensor.matmul(
                            ps[:, ci_, :cw],
                            wk8[:, 2 * j:2 * j + 2, m * 128:(m + 1) * 128],
                            hskv8[:, 2 * j:2 * j + 2, off:off + cw],
                            start=(j == 0), stop=(j == 2), perf_mode=DR)
                    nc.vector.tensor_scalar(
                        out=kT8[:, m, off:off + cw], in0=ps[:, ci_, :cw],
                        scalar1=1.0 / 64.0, scalar2=bk2[:, m:m + 1],
                        op0=OP.mult, op1=OP.add)

            def qproj_big(m, c):
                ps3 = pss.tile([128, 3, 512], F32, tag="sT",
                               name=f"psqb{m}_{c}")
                ps = ps3[:, 0, :]
                for j in range(3):
                    nc.tensor.matmul(
                        ps[:],
                        wq8[:, 2 * j:2 * j + 2, m * 128:(m + 1) * 128],
                        hsq8[:, 2 * j:2 * j + 2, c * 512:(c + 1) * 512],
                        start=(j == 0), stop=(j == 2), perf_mode=DR)
                nc.vector.tensor_scalar(
                    out=qT8[:, m, c * 512:(c + 1) * 512], in0=ps[:],
                    scalar1=1.0 / 512.0, scalar2=bq2[:, m:m + 1],
                    op0=OP.mult, op1=OP.add)

            def push_kproj(m, conv):
                for (off, cw) in kchunks:
                    def fn(m=m, off=off, cw=cw, conv=conv):
                        ps = psx.tile([128, 512], F32, tag="px",
                                      name=f"psk{m}_{off}")
                        for j in range(3):
                            nc.tensor.matmul(
                                ps[:, :cw],
                                wk8[:, 2 * j:2 * j + 2, m * 128:(m + 1) * 128],
                                hskv8[:, 2 * j:2 * j + 2, off:off + cw],
                                start=(j == 0), stop=(j == 2), perf_mode=DR)
                        conv.tensor_scalar(
                            out=kT8[:, m, off:off + cw], in0=ps[:, :cw],
                            scalar1=1.0 / 64.0, scalar2=bk2[:, m:m + 1],
                            op0=OP.mult, op1=OP.add)
                    pending.append((f"k{m}_{off}", fn))

            def push_qproj(m, conv):
                for c in range(2):
                    def fn(m=m, c=c, conv=conv):
                        co = c * 512
                        ps = psx.tile([128, 512], F32, tag="px",
                                      name=f"psq{m}_{c}")
                        for j in range(3):
                            nc.tensor.matmul(
                                ps[:],
                                wq8[:, 2 * j:2 * j + 2, m * 128:(m + 1) * 128],
                                hsq8[:, 2 * j:2 * j + 2, co:co + 512],
                                start=(j == 0), stop=(j == 2), perf_mode=DR)
                        conv.tensor_scalar(
                            out=qT8[:, m, co:co + 512], in0=ps[:],
                            scalar1=1.0 / 512.0, scalar2=bq2[:, m:m + 1],
                            op0=OP.mult, op1=OP.add)
                    pending.append((f"kq{m}" if c == 1 else f"q{m}_{c}", fn))

            pv0 = v_pad8[:].ap[0]
            vrow = (NH // 2) * 192

            def push_vproj(tb, ci, conv, label=None):
                def fn(tb=tb, ci=ci, conv=conv):
                    ps = psx.tile([128, 384], F32, tag="px",
                                  name=f"psv{tb}_{ci}")
                    for j in range(3):
                        nc.tensor.matmul(
                            ps[:],
                            hskv8[:, 2 * j:2 * j + 2, tb * 128:(tb + 1) * 128],
                            wv8[:, 2 * j:2 * j + 2, ci * 384:(ci + 1) * 384],
                            start=(j == 0), stop=(j == 2), perf_mode=DR)
                    dst = bass.AP(
                        tensor=v_pad8.tensor,
                        offset=v_pad8[:].offset + tb * vrow + ci * 576,
                        ap=[pv0, (192, 3), (128, 2), (1, 64)])
                    conv.tensor_scalar(
                        out=dst, in0=ps[:],
                        scalar1=m01[:, tb:tb + 1], scalar2=1.0 / 64.0,
                        op0=OP.mult, op1=OP.mult)
                pending.append((label or f"v{tb}_{ci}", fn))

            def vones(tb, conv):
                ones_dst = bass.AP(
                    tensor=v_pad8.tensor,
                    offset=v_pad8[:].offset + tb * vrow + 64,
                    ap=[pv0, (192, 6), (1, 64)])
                # ones col = 0.25 * m01
                conv.tensor_scalar(
                    out=ones_dst, in0=ones384[:],
                    scalar1=m01[:, tb:tb + 1], scalar2=0.25,
                    op0=OP.mult, op1=OP.mult)

            pT_tiles = {}
            pending = []          # queue of deferred psx rounds (closures)
            drained = set()       # labels fully drained

            def drain(n):
                k = 0
                while pending and k < n:
                    label, fn = pending.pop(0)
                    fn()
                    drained.add(label)
                    k += 1

            def drain_until(label):
                while pending and label not in drained:
                    lb, fn = pending.pop(0)
                    fn()
                    drained.add(lb)

            _slot = [0]

            def sc_exp_pair(hj, c):
                """scores + exp for a head pair (2hj, 2hj+1) x 512-query
                chunk. The two heads' K=64 score matmuls sit in row groups
                0-1 / 2-3 (base_partition 0 / 64), emitted adjacently so
                the PE runs them concurrently; outputs land in the two
                scores bufs."""
                co = c * 512
                drain_until(f"kq{hj}" if c == 1 else f"q{hj}_0")
                pTs = []
                for po in (0, 64):
                    h = 2 * hj + po // 64
                    pT = pss_sb.tile([128, nbk, 512], F8, tag="pT",
                                     name=f"pT{h}_{c}")
                    pT_tiles[(h, c)] = pT
                    pTs.append(pT)
                for g0 in range(0, nbk, 3):
                    g1 = min(g0 + 3, nbk)
                    for pi, po in enumerate((0, 64)):
                        ps = pss.tile([128, 3, 512], F32, tag="sT",
                                      name=f"sT{2 * hj + pi}_{c}_{g0}")
                        for i in range(g0, g1):
                            nc.tensor.matmul(
                                ps[:, i - g0, :],
                                kT8[po:po + 64, hj, i * 128:(i + 1) * 128],
                                qT8[po:po + 64, hj, co:co + 512])
                        nc.scalar.activation(
                            pTs[pi][:, g0:g1, :], ps[:, 0:g1 - g0, :],
                            AF.Exp, scale=1.0 / 256.0, bias=spb[:])
                        _slot[0] += 1
                        drain(2 if _slot[0] % 3 == 0 else 1)

            def ctx_head(h, c):
                """attn*V (rowsum via masked 0.25-ones col) + normalize."""
                hj = h // 2
                po = (h % 2) * 64
                co = c * 512
                drain_until(f"vci{0 if h < 6 else 1}")
                pT = pT_tiles.pop((h, c))
                vco = hj * 192 + po
                cps = psc.tile([128, 512], F32, tag="cT", name=f"cT{h}_{c}")
                for i2 in range(npair):
                    nc.tensor.matmul(
                        cps[:], v_pad8[:, 2 * i2:2 * i2 + 2, vco:vco + 128],
                        pT[:, 2 * i2:2 * i2 + 2, :],
                        start=(i2 == 0), stop=(ktail == 0 and i2 == npair - 1),
                        perf_mode=DR)
                if ktail:
                    nc.tensor.matmul(
                        cps[:], v_pad8[:, nbk - 1, vco:vco + 128],
                        pT[:, nbk - 1, :], start=(npair == 0), stop=True)
                rs = rsp.tile([128, 512], F32, tag="rs", name=f"rs{h}_{c}")
                nc.vector.reciprocal(
                    rs[po:po + 64, :], cps[64 - po:128 - po, :])
                nc.vector.tensor_tensor(
                    out=ctxT8[po:po + 64, hj, co:co + 512],
                    in0=cps[po:po + 64, :], in1=rs[po:po + 64, :],
                    op=OP.mult)

            def push_outproj(m, c, conv, pool=None, ptag="px", jorder=None):
                def fn(m=m, c=c, conv=conv, pool=pool, ptag=ptag,
                       jorder=jorder):
                    co = c * 512
                    pl = pool if pool is not None else psx
                    if ptag == "sT":
                        pst = pl.tile([128, 3, 512], F32, tag="sT",
                                      name=f"pso{m}_{c}")
                        ps = pst[:, 0, :]
                    else:
                        ps = pl.tile([128, 512], F32, tag=ptag,
                                     name=f"pso{m}_{c}")
                    js = jorder or (0, 1, 2)
                    for n_, j in enumerate(js):
                        nc.tensor.matmul(
                            ps[:],
                            wo8[:, 2 * j:2 * j + 2, m * 128:(m + 1) * 128],
                            ctxT8[:, 2 * j:2 * j + 2, co:co + 512],
                            start=(n_ == 0), stop=(n_ == 2), perf_mode=DR)
                    if conv is nc.scalar:
                        # Identity is in every ACT table set: no reload.
                        nc.scalar.activation(
                            outT[:, m, co:co + 512], ps[:], AF.Identity,
                            scale=1.0 / 4096.0, bias=bo2[:, m:m + 1])
                    else:
                        conv.tensor_scalar(
                            out=outT[:, m, co:co + 512], in0=ps[:],
                            scalar1=1.0 / 4096.0, scalar2=bo2[:, m:m + 1],
                            op0=OP.mult, op1=OP.add)
                pending.append((f"op{c}_{m}", fn))

            # ---- layernorm, split into stats / batched-rstd / affine ----
            y_tiles = {}

            def ln_transpose_add(tb, pt_pool, ptag):
                pt = pt_pool.tile([128, 6, 128], BF16, tag=ptag,
                                  name=f"pt{tb}")
                for m in range(6):
                    nc.tensor.transpose(
                        pt[:, m, :], outT[:, m, tb * 128:(tb + 1) * 128],
                        identb[:])
                y = pdl.tile([128, H], F32, tag=f"y{tb % 2}", name=f"y{tb}")
                y_tiles[tb] = y
                # pt is PSUM: GPSIMD cannot read PSUM, this add must be DVE
                nc.vector.tensor_tensor(
                    out=y[:], in0=pt[:].rearrange("p a b -> p (a b)"),
                    in1=hs_res[:, tb, :], op=OP.add)
                return y

            def ln_stats_dve(tb):
                """mid-stream: stats on DVE (ACT is saturated by exp)."""
                y = ln_transpose_add(tb, psx, "px")
                stats = pdl.tile([128, 3, 6], F32, tag="st", name=f"st{tb}")
                yv = y[:].rearrange("p (n f) -> p n f", f=256)
                for g in range(3):
                    nc.vector.bn_stats(out=stats[:, g, :], in_=yv[:, g, :])
                nc.vector.bn_aggr(
                    out=mvs[:, tb:tb + 1, :].rearrange("p a b -> p (a b)"),
                    in_=stats[:])

            def ln_stats_act(tb, pt_pool, ptag):
                """tail: sums via ACT accum passes (ACT is idle there)."""
                y = ln_transpose_add(tb, pt_pool, ptag)
                nc.scalar.activation(
                    dump[:], y[:], AF.Identity,
                    accum_out=sms[:, tb:tb + 1, 0:1].rearrange(
                        "p a b -> p (a b)"))
                nc.scalar.activation(
                    dump[:], y[:], AF.Square,
                    accum_out=sms[:, tb:tb + 1, 1:2].rearrange(
                        "p a b -> p (a b)"))

            def _rsqrt_nr(p, ve):
                """rstds[:, 2p:2p+2] = rsqrt(ve) : bit-trick + 2 Newton."""
                yt = pdl.tile([128, 2], F32, tag="yt", name=f"yt{p}")
                nc.vector.tensor_scalar(
                    out=yt[:].bitcast(I32), in0=ve[:].bitcast(I32),
                    scalar1=1, scalar2=None, op0=OP.logical_shift_right)
                nc.vector.tensor_tensor(
                    out=yt[:].bitcast(I32), in0=magicf[:].bitcast(I32),
                    in1=yt[:].bitcast(I32), op=OP.subtract)
                t = pdl.tile([128, 2], F32, tag="t", name=f"t{p}")
                w = pdl.tile([128, 2], F32, tag="w", name=f"w{p}")
                for it in range(2):
                    dsty = rstds[:, 2 * p:2 * p + 2] if it == 1 else yt[:]
                    nc.vector.tensor_tensor(out=t[:], in0=ve[:], in1=yt[:],
                                            op=OP.mult)
                    nc.vector.tensor_tensor(out=t[:], in0=t[:], in1=yt[:],
                                            op=OP.mult)
                    nc.vector.tensor_scalar(
                        out=w[:], in0=t[:], scalar1=-0.5, scalar2=1.5,
                        op0=OP.mult, op1=OP.add)
                    nc.vector.tensor_tensor(out=dsty, in0=w[:], in1=yt[:],
                                            op=OP.mult)

            def ln_rstd_pair_mv(p):
                """pair rstd from bn_aggr's (mean, var) in mvs."""
                vv = mvs[:, 2 * p:2 * p + 2, 1:2].rearrange(
                    "p a b -> p (a b)")
                ve = pdl.tile([128, 2], F32, tag="ve", name=f"ve{p}")
                nc.vector.tensor_scalar(
                    out=ve[:], in0=vv, scalar1=LN_EPS, scalar2=None,
                    op0=OP.add)
                _rsqrt_nr(p, ve)

            def ln_rstd_pair_sums(p):
                """pair rstd + mean from ACT (sum, sumsq) in sms."""
                sy = sms[:, 2 * p:2 * p + 2, 0:1].rearrange(
                    "p a b -> p (a b)")
                sq = sms[:, 2 * p:2 * p + 2, 1:2].rearrange(
                    "p a b -> p (a b)")
                mm = mvs[:, 2 * p:2 * p + 2, 0:1].rearrange(
                    "p a b -> p (a b)")
                nc.vector.tensor_scalar(
                    out=mm, in0=sy, scalar1=1.0 / H, scalar2=None,
                    op0=OP.mult)
                ve = pdl.tile([128, 2], F32, tag="ve", name=f"ve{p}")
                nc.vector.tensor_scalar(
                    out=ve[:], in0=sq, scalar1=1.0 / H, scalar2=LN_EPS,
                    op0=OP.mult, op1=OP.add)
                m2 = pdl.tile([128, 2], F32, tag="m2", name=f"m2{p}")
                nc.vector.scalar_tensor_tensor(
                    out=m2[:], in0=mm, scalar=-1.0, in1=mm,
                    op0=OP.mult, op1=OP.mult)
                nc.vector.tensor_tensor(
                    out=ve[:], in0=ve[:], in1=m2[:], op=OP.add)
                _rsqrt_nr(p, ve)

            def ln_aff(tb, e0, e1, e2):
                y = y_tiles.pop(tb)
                nmr = pdl.tile([128, 1], F32, tag="nmr", name=f"nmr{tb}")
                nc.vector.scalar_tensor_tensor(
                    out=nmr[:], in0=mvs[:, tb:tb + 1, 0:1].rearrange(
                        "p a b -> p (a b)"),
                    scalar=-1.0, in1=rstds[:, tb:tb + 1],
                    op0=OP.mult, op1=OP.mult)
                yn = pdl.tile([128, H], F32, tag="yn", name=f"yn{tb}")
                e0.tensor_scalar(
                    out=yn[:], in0=y[:], scalar1=rstds[:, tb:tb + 1],
                    scalar2=nmr[:], op0=OP.mult, op1=OP.add)
                e1.tensor_tensor(out=yn[:], in0=yn[:], in1=gam[:],
                                 op=OP.mult)
                e2.tensor_tensor(out=yn[:], in0=yn[:], in1=bet[:],
                                 op=OP.add)
                nc.sync.dma_start(y_d.ap()[tb * 128:(tb + 1) * 128, :], yn[:])

            def push_ln_pair(t0):
                p = t0 // 2
                pending.append((f"lns{t0}", lambda tb=t0: ln_stats_dve(tb)))
                pending.append((f"lns{t0 + 1}",
                                lambda tb=t0 + 1: ln_stats_dve(tb)))
                pending.append((f"lnr{p}",
                                lambda p=p: ln_rstd_pair_mv(p)))
                pending.append((f"lna{t0}",
                                lambda tb=t0: ln_aff(tb, nc.gpsimd,
                                                     nc.vector, nc.gpsimd)))
                pending.append((f"lna{t0 + 1}",
                                lambda tb=t0 + 1: ln_aff(
                                    tb, nc.gpsimd, nc.vector, nc.gpsimd)))

            with tc.tile_pool(name="pTp", bufs=14) as pss_sb, \
                 tc.tile_pool(name="rsP", bufs=2) as rsp, \
                 tc.tile_pool(name="phD", bufs=2) as pdl:
                # ---- prologue, in dependency order of the DMAs: kproj m0
                # (hskv8 lands first), then qproj (m0, c0); qproj (m0, c1)
                # is deferred so its late hsq8 half never blocks the PE
                # FIFO ahead of the first scores ----
                kproj_big(0)
                qproj_big(0, 0)
                drained.add("q0_0")

                def _q01():
                    # scheduler-side floor: keep this out of the startup
                    # PE FIFO (its hsq8 half lands late)
                    with tc.tile_wait_until(0.010):
                        qproj_big(0, 1)
                pending.append(("kq0", _q01))
                for tb in range(nbk):
                    vones(tb, nc.gpsimd)
                # ---- deferred psx rounds, drained ~1.33/score-group into
                # the gaps of the exp stream; K/Q first (hard deadlines),
                # then V (ctx is deferred until its V half is written) ----
                push_kproj(1, nc.vector)
                push_qproj(1, nc.vector)
                push_kproj(2, nc.vector)
                push_qproj(2, nc.vector)
                for tb in range(nbk):
                    push_vproj(tb, 0, nc.vector,
                               label=("vci0" if tb == nbk - 1 else None))
                push_kproj(3, nc.vector)
                push_qproj(3, nc.vector)
                push_kproj(4, nc.vector)
                push_qproj(4, nc.vector)
                push_kproj(5, nc.vector)
                push_qproj(5, nc.vector)
                for tb in range(nbk):
                    push_vproj(tb, 1, nc.vector,
                               label=("vci1" if tb == nbk - 1 else None))

                # c=1 pairs run 4,5 first (heads 8..11) so outproj's final
                # accumulation step (j=1: heads 4..7) covers the
                # last-finished quartet.
                pair_order = ([(hj, 0) for hj in range(6)]
                              + [(hj, 1) for hj in (4, 5, 0, 1, 2, 3)])
                ctx_todo = [(2 * hj + u, c) for (hj, c) in pair_order
                            for u in (0, 1)]

                def ctx_ready(h, c):
                    return (f"vci{0 if h < 6 else 1}" in drained
                            and (h, c) in pT_tiles)

                op0_done = False
                for (hj, c) in pair_order:
                    sc_exp_pair(hj, c)
                    for _ in range(3):
                        if ctx_todo and ctx_ready(*ctx_todo[0]):
                            ctx_head(*ctx_todo.pop(0))
                        else:
                            break
                    if not op0_done and not any(cc == 0 for (_, cc) in ctx_todo):
                        for m in range(6):
                            push_outproj(m, 0, nc.vector)
                        push_ln_pair(0)
                        push_ln_pair(2)
                        op0_done = True
                # ---- tail: pipeline outproj c=1 over three PSUM tiles
                # (psx + both scores bufs) with converts on DVE/ACT ----
                while ctx_todo:
                    ctx_head(*ctx_todo.pop(0))
                drain(len(pending))
                jo = (2, 0, 1)
                for m in range(6):
                    if m % 3 == 0:
                        push_outproj(m, 1, nc.vector, jorder=jo)
                    else:
                        push_outproj(m, 1, nc.scalar, pool=pss, ptag="sT",
                                     jorder=jo)
                drain(len(pending))
                ln_stats_act(4, psc, "cT")
                ln_stats_act(5, psx, "px")
                ln_rstd_pair_sums(2)
                ln_stats_act(6, psc, "cT")
                ln_stats_act(7, psx, "px")
                ln_aff(4, nc.vector, nc.gpsimd, nc.vector)
                ln_rstd_pair_sums(3)
                ln_aff(5, nc.gpsimd, nc.vector, nc.gpsimd)
                ln_aff(6, nc.vector, nc.gpsimd, nc.vector)
                ln_aff(7, nc.gpsimd, nc.vector, nc.gpsimd)

    nc.compile()
    return nc


def _make_in_maps(inputs, idxs, skp):
    import ml_dtypes
    F8NP = ml_dtypes.float8_e4m3
    BF16NP = ml_dtypes.bfloat16

    hs = np.ascontiguousarray(np.asarray(inputs["hidden_states"], np.float32))
    Wq, Wk, Wv, Wo = (np.asarray(inputs[k], np.float32)
                      for k in ("Wq", "Wk", "Wv", "Wo"))
    bq, bk, bv, bo = (np.asarray(inputs[k], np.float32)
                      for k in ("bq", "bk", "bv", "bo"))
    wqT8 = np.ascontiguousarray((64.0 * Wq.T).astype(F8NP))
    wkT8 = np.ascontiguousarray((64.0 * Wk.T).astype(F8NP))
    wvT8 = np.ascontiguousarray((64.0 * Wv.T).astype(F8NP))
    woT8 = np.ascontiguousarray((64.0 * Wo.T).astype(F8NP))
    bq2 = np.ascontiguousarray((2.0 * bq).reshape(6, 128).T)
    bk2 = np.ascontiguousarray((16.0 * bk).reshape(6, 128).T)
    bo_eff = bo + Wo @ bv
    bo2 = np.ascontiguousarray(bo_eff.reshape(6, 128).T)
    gam = np.asarray(inputs["ln_gamma"], np.float32).reshape(1, H)
    bet = np.asarray(inputs["ln_beta"], np.float32).reshape(1, H)

    in_maps = []
    for core in range(N_CORES):
        b, sh = divmod(core, 2)
        ix = idxs[b]
        hsk = np.zeros((skp, H), np.float32)
        hsk[:len(ix)] = hs[b][ix]
        m01 = np.zeros(skp, np.float32)
        m01[:len(ix)] = 1.0
        hq = hs[b, sh * SQ:(sh + 1) * SQ]
        in_maps.append({
            "hsT_kv8": np.ascontiguousarray((16.0 * hsk.T).astype(F8NP)),
            "hsT_q8": np.ascontiguousarray((16.0 * hq.T).astype(F8NP)),
            "hs_q": np.ascontiguousarray(hq.astype(BF16NP)),
            "wqT8": wqT8, "wkT8": wkT8, "wvT8": wvT8, "woT8": woT8,
            "bq2": bq2, "bk2": bk2, "bo2": bo2,
            "m01": np.ascontiguousarray(m01.reshape(skp // 128, 128).T),
            "gam": gam, "bet": bet,
        })
    return in_maps


def kernel(hidden_states, Wq, bq, Wk, bk, Wv, bv, Wo, bo, dim_biases,
           ln_gamma, ln_beta, attention_mask, dimension_idx):
    hs = np.asarray(hidden_states, dtype=np.float32)
    mask = np.asarray(attention_mask)
    B, S, _ = hs.shape

    idxs = [np.nonzero(mask[b] != 0)[0] for b in range(B)]
    skp = max(256, ((max(len(ix) for ix in idxs) + 127) // 128) * 128)

    if skp not in _CACHE:
        _CACHE[skp] = _build(skp)
    nc = _CACHE[skp]

    in_maps = _make_in_maps(
        {"hidden_states": hs, "Wq": Wq, "Wk": Wk, "Wv": Wv, "Wo": Wo,
         "bq": bq, "bk": bk, "bv": bv, "bo": bo,
         "ln_gamma": ln_gamma, "ln_beta": ln_beta}, idxs, skp)

    res = run_bass_kernel_spmd(nc, in_maps, list(range(N_CORES)))

    out = np.empty((B, S, H), np.float32)
    for core in range(N_CORES):
        b, sh = divmod(core, 2)
        out[b, sh * SQ:(sh + 1) * SQ] = res.results[core]["y_out"]
    return out
